# revision 31
# baseline (speedup 1.0000x reference)
"""Trainium2 Bass kernel for a 3-layer GCN encoder with global max pool.

Strategy (8 NeuronCores, SPMD, 4 launches):
  - Nodes partitioned graph-wise across cores (graph g -> core g//64).
  - GCN normalization factored out: with s = 1/sqrt(deg), table rows are
    pre-scaled (s*h), so aggregation is an unweighted gather+sum and all
    per-layer scaling collapses into per-(partition,tile) activation scales
    (relu(s*y) == s*relu(y) since s > 0).
  - Each layer's full node table lives in HBM per core.  Per-core tables are
    HOT/COLD ordered by that core's source-usage counts so the int16 gather
    index window [0, 32768) covers ~92% of edges; the cold window needs only
    ~60 padded slots total.  Destination nodes are sorted by (kB, kA) so the
    padded-CSR tiles are tight, with exact per-tile slot widths (no group-max
    padding).
  - dma_gather (padded CSR, one 256B descriptor per edge) + strided DVE
    tensor_reduce per 128-node tile.  One cold-window gather call per launch,
    hot-window gather calls per group of G tiles, heaviest groups first so
    the post-gather tail is short.  Everything after the per-tile reduce
    (self add, scale, bias, relu, the layer-2 W2/W3 matmul chain, output
    stores) is emitted per tile / per group so it pipelines under the
    remaining gather DMA.
  - Matmuls in bf16; PSUM<->SBUF moves, scaling, bias and relu are fused into
    single per-tile Activation-engine ops.  Launch outputs are contiguous
    strip stores; the host does de-striping/permutation between launches
    (the "AllGather").
  - Global max pool (launch 4): store the layer-3 strip per group, gather
    per-graph padded node lists (2 SBUF partitions per graph), reduce-max,
    PE transpose, pairwise max.
"""

import numpy as np

N_NODES = 50000
N_EDGES = 600000
IN_DIM = 128
HID = 64
N_GRAPHS = 512
C = 8           # cores
P = 128         # partitions
GPC = N_GRAPHS // C
SPLIT = 32768   # int16 index range per dma_gather call
G = 4           # tiles per hot-window gather call


def _pack_idx16(flat):
    """[num] int array -> wrapped [128, num//16] int16 (16-wrapped, 8x repl)."""
    num = flat.shape[0]
    assert num % 16 == 0
    arr = flat.reshape(num // 16, 16).T.astype(np.int16)   # [16, num//16]
    return np.tile(arr, (8, 1))                            # [128, num//16]


def _slots(vtile, vpart):
    """Running slot index within each (tile, partition) group."""
    order = np.lexsort((vpart, vtile))
    key = vtile[order] * P + vpart[order]
    newgrp = np.concatenate([[True], key[1:] != key[:-1]])
    gsp = np.nonzero(newgrp)[0]
    slot = np.arange(len(key)) - gsp[np.cumsum(newgrp) - 1]
    return order, slot


# --------------------------------------------------------------------------
# Host-side preprocessing: sharding, permutations, padded CSR index arrays.
# --------------------------------------------------------------------------

KBCAP = 3   # cold-edge-count band values: 0, 1, 2, and "3 or more"


def _host_prep(edge_index, batch):
    N = N_NODES
    src = np.asarray(edge_index[0], dtype=np.int64)
    dst = np.asarray(edge_index[1], dtype=np.int64)
    batch = np.asarray(batch, dtype=np.int64)

    indeg = np.bincount(dst, minlength=N)
    s = (1.0 / np.sqrt((indeg + 1).astype(np.float64))).astype(np.float64)

    core_of_node = batch // GPC
    dst_core = core_of_node[dst]
    NTAB = N + 2

    # pass 1: per-core hot/cold table order and per-dst hot/cold edge counts
    percore = []
    band_sizes = np.zeros((C, KBCAP + 1), np.int64)
    for c in range(C):
        m = dst_core == c
        s_c = src[m]
        d_c = dst[m]
        u = np.bincount(s_c, minlength=N)
        tabperm = np.argsort(-u, kind="stable")
        tabrow = np.empty(N, np.int64)
        tabrow[tabperm] = 1 + np.arange(N)
        erow = tabrow[s_c]
        lowm = erow < SPLIT
        nodes_c = np.nonzero(core_of_node == c)[0]
        kA = np.bincount(d_c[lowm], minlength=N)[nodes_c]
        kB = np.bincount(d_c[~lowm], minlength=N)[nodes_c]
        kb = np.minimum(kB, KBCAP)
        band_sizes[c] = np.bincount(kb, minlength=KBCAP + 1)
        percore.append((nodes_c, d_c, erow, lowm, kA, kB, kb, tabperm))

    # band-aligned tile layout: band b starts at the same tile on every core
    tiles_b = [int(-(-band_sizes[:, b].max() // P)) for b in range(KBCAP + 1)]
    bstart = np.concatenate([[0], np.cumsum(tiles_b)])
    T = int(bstart[-1])
    Npc = T * P

    rowmaps = []        # per core: node id at local row, -1 for pad cells
    tabperms = []       # per core: global node at table row 1+i
    e_cell = []         # per core: (erow, p, t, lowmask) for its in-edges
    for c in range(C):
        nodes_c, d_c, erow, lowm, kA, kB, kb, tabperm = percore[c]
        rowmap = np.full(Npc, -1, np.int64)
        row_of = np.full(N, -1, np.int64)
        for b in range(KBCAP + 1):
            sel = np.nonzero(kb == b)[0]
            order = sel[np.argsort(-kA[sel], kind="stable")]
            rows = bstart[b] * P + np.arange(len(order))
            rowmap[rows] = nodes_c[order]
            row_of[nodes_c[order]] = rows
        lp = row_of[d_c]
        rowmaps.append(rowmap)
        tabperms.append(tabperm)
        e_cell.append((erow, lp % P, lp // P, lowm))

    # per-tile slot widths (max over cores and partitions)
    DA_t = np.zeros(T, np.int64)
    DB_t = np.zeros(T, np.int64)
    for c in range(C):
        erow, vp, vt, lowm = e_cell[c]
        cntA = np.zeros((T, P), np.int64)
        np.add.at(cntA, (vt[lowm], vp[lowm]), 1)
        cntB = np.zeros((T, P), np.int64)
        np.add.at(cntB, (vt[~lowm], vp[~lowm]), 1)
        DA_t = np.maximum(DA_t, cntA.max(axis=1))
        DB_t = np.maximum(DB_t, cntB.max(axis=1))
    prefA = np.concatenate([[0], np.cumsum(DA_t)])
    prefB = np.concatenate([[0], np.cumsum(DB_t)])
    ncolsB = int(prefB[-1])
    padB = NTAB - 1 - SPLIT

    ngroups = -(-T // G)
    groups = []          # (col16_start, ncolsA, t0, gsz)
    col16 = 8 * ncolsB   # B block first
    for g in range(ngroups):
        t0 = g * G
        gsz = min(G, T - t0)
        ncA = int(prefA[t0 + gsz] - prefA[t0])
        groups.append((col16, ncA, t0, gsz))
        col16 += 8 * ncA
    COLS16 = col16

    idx16 = np.zeros((C, P, COLS16), np.int16)
    for c in range(C):
        erow, vp, vt, lowm = e_cell[c]
        # hot side
        er, p_, t_ = erow[lowm], vp[lowm], vt[lowm]
        order, slot = _slots(t_, p_)
        er, p_, t_ = er[order], p_[order], t_[order]
        g_ = t_ // G
        colg = (prefA[t_] - prefA[(t_ // G) * G]) + slot
        for g in range(ngroups):
            c16, ncA, t0, gsz = groups[g]
            if ncA == 0:
                continue
            flat = np.zeros(P * ncA, np.int64)
            m = g_ == g
            flat[colg[m] * P + p_[m]] = er[m]
            idx16[c][:, c16:c16 + 8 * ncA] = _pack_idx16(flat)
        # cold side
        er, p_, t_ = erow[~lowm] - SPLIT, vp[~lowm], vt[~lowm]
        order, slot = _slots(t_, p_)
        er, p_, t_ = er[order], p_[order], t_[order]
        if ncolsB:
            flat = np.full(P * ncolsB, padB, np.int64)
            flat[(prefB[t_] + slot) * P + p_] = er
            idx16[c][:, 0:8 * ncolsB] = _pack_idx16(flat)

    # per-(partition,tile) normalization scale strips: s, s^2
    dinvT = np.zeros((2, C, P, T), np.float64)
    for c in range(C):
        pad = np.zeros(Npc, np.float64)
        valid = rowmaps[c] >= 0
        pad[valid] = s[rowmaps[c][valid]]
        cell = pad.reshape(T, P).T                 # [P, T]
        dinvT[0, c] = cell
        dinvT[1, c] = cell ** 2
    dinvT = dinvT.astype(np.float32)

    # pooling CSR: graph local slot l -> partitions 2g, 2g+1 (alternating);
    # table row of node at local cell (p, t) in the stored strip is 1 + p*T + t
    Dp = 0
    pool_rows = []
    for c in range(C):
        loc = np.nonzero(rowmaps[c] >= 0)[0]
        gl = batch[rowmaps[c][loc]] % GPC
        order = np.lexsort((loc, gl))
        ogl, oloc = gl[order], loc[order]
        newg = np.concatenate([[True], ogl[1:] != ogl[:-1]])
        gsp = np.nonzero(newg)[0]
        gslot = np.arange(len(ogl)) - gsp[np.cumsum(newg) - 1]
        ppart = 2 * ogl + (gslot % 2)
        pslot = gslot // 2
        Dp = max(Dp, int(pslot.max()) + 1)
        pool_rows.append((ppart, pslot, 1 + (oloc % P) * T + (oloc // P)))
    pool16 = np.zeros((C, P, 8 * Dp), np.int16)
    for c in range(C):
        ppart, pslot, prow = pool_rows[c]
        flat = np.zeros(P * Dp, np.int64)          # pad -> row 0 (-inf row)
        flat[pslot * P + ppart] = prow
        pool16[c] = _pack_idx16(flat)

    meta = dict(T=T, Npc=Npc, NTAB=NTAB, COLS16=COLS16, ncolsB=ncolsB,
                prefA=prefA.tolist(), prefB=prefB.tolist(),
                DA_t=DA_t.tolist(), DB_t=DB_t.tolist(),
                groups=groups, Dp=Dp)
    return dict(idx16=idx16, pool16=pool16, dinvT=dinvT,
                rowmaps=rowmaps, tabperms=tabperms, s=s, meta=meta)


# --------------------------------------------------------------------------
# Bass programs (4 launches)
# --------------------------------------------------------------------------

def _mk_bass():
    import concourse.bacc as bacc
    return bacc.Bacc(None)


def _emit_agg(nc, meta, TBL_d, IDX_s, Bbuf, agg_strip, SELF_s, msgp, ztp,
              finish_tile, finish_group):
    """Gather + reduce + self-add per tile, heaviest groups first.

    finish_tile(t, sl): emit the per-tile tail (scale/relu/matmuls...)
    finish_group(t0, gsz): emit the per-group output store
    """
    import concourse.mybir as mybir
    f32 = mybir.dt.float32
    Alu = mybir.AluOpType
    Axis = mybir.AxisListType
    NTAB = meta["NTAB"]
    ncolsB = meta["ncolsB"]
    prefA, prefB = meta["prefA"], meta["prefB"]
    DA_t, DB_t = meta["DA_t"], meta["DB_t"]

    if ncolsB:
        nc.gpsimd.dma_gather(
            out_ap=Bbuf[:].rearrange("p (d f) -> p d f", f=HID),
            in_ap=TBL_d[SPLIT:NTAB, :],
            idxs_ap=IDX_s[:, 0:8 * ncolsB],
            num_idxs=P * ncolsB,
            num_idxs_reg=P * ncolsB,
            elem_size=HID,
            single_packet=False,
        )
    order = sorted(meta["groups"], key=lambda g: -g[1])   # heavy first
    for (c16, ncA, t0, gsz) in order:
        msg = None
        if ncA:
            msg = msgp.tile([P, ncA * HID], f32, tag="msg", name="msg")
            nc.gpsimd.dma_gather(
                out_ap=msg[:].rearrange("p (d f) -> p d f", f=HID),
                in_ap=TBL_d[0:SPLIT, :],
                idxs_ap=IDX_s[:, c16:c16 + 8 * ncA],
                num_idxs=P * ncA,
                num_idxs_reg=P * ncA,
                elem_size=HID,
                single_packet=False,
            )
        for t in range(t0, t0 + gsz):
            sl = agg_strip[:, t * HID:(t + 1) * HID]
            self_sl = SELF_s[:, t * HID:(t + 1) * HID]
            da, db = DA_t[t], DB_t[t]
            a0 = prefA[t] - prefA[t0]
            if da:
                nc.vector.tensor_reduce(
                    out=sl,
                    in_=msg[:, a0 * HID:(a0 + da) * HID]
                        .rearrange("p (d f) -> p f d", f=HID),
                    axis=Axis.X,
                    op=Alu.add,
                )
            if db:
                b0 = prefB[t]
                bap = Bbuf[:, b0 * HID:(b0 + db) * HID] \
                    .rearrange("p (d f) -> p f d", f=HID)
                if da:
                    tmp = ztp.tile([P, HID], f32, tag="btmp", name="btmp")
                    nc.vector.tensor_reduce(out=tmp[:], in_=bap,
                                            axis=Axis.X, op=Alu.add)
                    nc.vector.tensor_tensor(out=sl, in0=sl, in1=tmp[:],
                                            op=Alu.add)
                else:
                    nc.vector.tensor_reduce(out=sl, in_=bap,
                                            axis=Axis.X, op=Alu.add)
            if da or db:
                nc.vector.tensor_tensor(out=sl, in0=sl, in1=self_sl,
                                        op=Alu.add)
            else:
                nc.vector.tensor_copy(sl, self_sl)
            finish_tile(t, sl)
        finish_group(t0, gsz)


def _prog_tables(meta):
    """Launch 1: T1 strip = (s*X) @ W1 for this core's nodes (s folded into
    X on the host)."""
    import concourse.mybir as mybir
    import concourse.tile as tile

    T, Npc = meta["T"], meta["Npc"]
    f32 = mybir.dt.float32
    bf16 = mybir.dt.bfloat16
    Act = mybir.ActivationFunctionType
    nc = _mk_bass()

    XT_d = nc.dram_tensor("XT", [IN_DIM, Npc], bf16, kind="ExternalInput")
    W1_d = nc.dram_tensor("W1", [IN_DIM, HID], bf16, kind="ExternalInput")
    OUTS_d = nc.dram_tensor("OUTS", [P, T * HID], bf16,
                            kind="ExternalOutput")

    NCH = 4
    TC = -(-T // NCH)

    with tile.TileContext(nc, num_cores=C) as tc:
        with (
            tc.tile_pool(name="const", bufs=1) as const,
            tc.tile_pool(name="psum", bufs=4, space="PSUM") as psp,
        ):
            W1_s = const.tile([IN_DIM, HID], bf16)
            nc.sync.dma_start(W1_s[:], W1_d[:])
            chunks = []
            for ch in range(NCH):
                t0 = ch * TC
                tn = min(TC, T - t0)
                if tn <= 0:
                    break
                xt = const.tile([IN_DIM, tn * P], bf16, name=f"xt{ch}")
                nc.sync.dma_start(xt[:], XT_d[:, t0 * P:(t0 + tn) * P])
                chunks.append((t0, tn, xt))
            strip = const.tile([P, T * HID], bf16)
            for (t0, tn, xt) in chunks:
                for i in range(0, tn, 2):
                    t = t0 + i
                    n2 = min(2, tn - i)
                    ps = psp.tile([P, 2 * HID], f32, tag="ps", name="ps")
                    for j in range(n2):
                        nc.tensor.matmul(
                            ps[:, j * HID:(j + 1) * HID],
                            lhsT=xt[:, (i + j) * P:(i + j + 1) * P],
                            rhs=W1_s[:], start=True, stop=True)
                    nc.scalar.activation(
                        strip[:, t * HID:(t + n2) * HID],
                        ps[:, 0:n2 * HID], Act.Copy)
                nc.sync.dma_start(OUTS_d[:, t0 * HID:(t0 + tn) * HID],
                                  strip[:, t0 * HID:(t0 + tn) * HID])
    nc.compile()
    return nc


def _prog_layer(meta, layer):
    """Launches 2/3: aggregate TBL -> next table strip.

    layer=1: out = relu(s^2 * (Agg(T1) + SELFB)),  SELFB = self + b1/s^2
    layer=2: out = s * (h2 @ W3), h2 = relu((s*(Agg(T2)+self)) @ W2 + b2)
    """
    import concourse.mybir as mybir
    import concourse.tile as tile
    from concourse.masks import make_identity

    T, Npc, NTAB = meta["T"], meta["Npc"], meta["NTAB"]
    COLS16 = meta["COLS16"]
    ncolsB = meta["ncolsB"]
    f32 = mybir.dt.float32
    bf16 = mybir.dt.bfloat16
    i16 = mybir.dt.int16
    Alu = mybir.AluOpType
    Act = mybir.ActivationFunctionType
    nc = _mk_bass()

    TBL_d = nc.dram_tensor("TBL", [NTAB, HID], f32, kind="ExternalInput")
    SELF_d = nc.dram_tensor("SELF", [P, T * HID], bf16, kind="ExternalInput")
    DINV_d = nc.dram_tensor("DINV", [P, T], f32, kind="ExternalInput")
    IDX_d = nc.dram_tensor("IDX16", [P, COLS16], i16, kind="ExternalInput")
    OUTS_d = nc.dram_tensor("OUTS", [P, T * HID], bf16,
                            kind="ExternalOutput")
    if layer == 2:
        W2_d = nc.dram_tensor("W2", [HID, 2 * HID], bf16, kind="ExternalInput")
        W3_d = nc.dram_tensor("W3", [2 * HID, HID], bf16, kind="ExternalInput")
        B2_d = nc.dram_tensor("B2", [P, 1], f32, kind="ExternalInput")

    with tile.TileContext(nc, num_cores=C) as tc:
        with (
            tc.tile_pool(name="const", bufs=1) as const,
            tc.tile_pool(name="msg", bufs=3) as msgp,
            tc.tile_pool(name="zt", bufs=4) as ztp,
            tc.tile_pool(name="psum", bufs=2, space="PSUM") as psp,
        ):
            IDX_s = const.tile([P, COLS16], i16)
            if ncolsB:
                nc.sync.dma_start(IDX_s[:, 0:8 * ncolsB],
                                  IDX_d[:, 0:8 * ncolsB])
                nc.sync.dma_start(IDX_s[:, 8 * ncolsB:],
                                  IDX_d[:, 8 * ncolsB:])
            else:
                nc.sync.dma_start(IDX_s[:], IDX_d[:])
            DINV_s = const.tile([P, T], f32)
            nc.sync.dma_start(DINV_s[:], DINV_d[:])
            SELF_s = const.tile([P, T * HID], bf16)
            nc.sync.dma_start(SELF_s[:], SELF_d[:])
            if layer == 2:
                W2_s = const.tile([HID, 2 * HID], bf16)
                nc.sync.dma_start(W2_s[:], W2_d[:])
                W3_s = const.tile([2 * HID, HID], bf16)
                nc.sync.dma_start(W3_s[:], W3_d[:])
                B2_s = const.tile([P, 1], f32)
                nc.sync.dma_start(B2_s[:], B2_d[:])
                ident = const.tile([P, P], f32)
                make_identity(nc, ident[:])
                ident_bf = const.tile([P, P], bf16)
                nc.vector.tensor_copy(ident_bf[:], ident[:])
                aggbf = const.tile([P, T * HID], bf16)
            agg_strip = const.tile([P, T * HID], f32)
            out_strip = const.tile([P, T * HID], bf16)
            Bbuf = const.tile([P, max(ncolsB, 1) * HID], f32)

            if layer == 1:
                def finish_tile(t, sl):
                    nc.scalar.activation(
                        out_strip[:, t * HID:(t + 1) * HID], sl,
                        Act.Relu, scale=DINV_s[:, t:t + 1])
            else:
                def finish_tile(t, sl):
                    nc.scalar.activation(
                        aggbf[:, t * HID:(t + 1) * HID], sl,
                        Act.Copy, scale=DINV_s[:, t:t + 1])
                    psT = psp.tile([HID, P], bf16, tag="psT", name="psT")
                    nc.tensor.transpose(
                        psT[:], aggbf[:, t * HID:(t + 1) * HID], ident_bf[:])
                    zT = ztp.tile([HID, P], bf16, tag="zT", name="zT")
                    nc.scalar.activation(zT[:], psT[:], Act.Copy)
                    h2T = psp.tile([2 * HID, P], f32, tag="h2T", name="h2T")
                    nc.tensor.matmul(h2T[:], lhsT=W2_s[:], rhs=zT[:],
                                     start=True, stop=True)
                    h2Tbf = ztp.tile([2 * HID, P], bf16, tag="h2Tbf",
                                     name="h2Tbf")
                    nc.scalar.activation(h2Tbf[:], h2T[:], Act.Relu,
                                         bias=B2_s[:, 0:1])
                    ps3 = psp.tile([P, HID], f32, tag="ps3", name="ps3")
                    nc.tensor.matmul(ps3[:], lhsT=h2Tbf[:], rhs=W3_s[:],
                                     start=True, stop=True)
                    nc.vector.tensor_tensor(
                        out=out_strip[:, t * HID:(t + 1) * HID],
                        in0=ps3[:],
                        in1=DINV_s[:, t:t + 1].to_broadcast([P, HID]),
                        op=Alu.mult)

            def finish_group(t0, gsz):
                nc.sync.dma_start(
                    OUTS_d[:, t0 * HID:(t0 + gsz) * HID],
                    out_strip[:, t0 * HID:(t0 + gsz) * HID])

            _emit_agg(nc, meta, TBL_d, IDX_s, Bbuf, agg_strip, SELF_s,
                      msgp, ztp, finish_tile, finish_group)
    nc.compile()
    return nc


def _prog_final(meta):
    """Launch 4: layer-3 aggregation + bias, then global max pool."""
    import concourse.mybir as mybir
    import concourse.tile as tile
    from concourse.masks import make_identity

    T, Npc, NTAB, Dp = meta["T"], meta["Npc"], meta["NTAB"], meta["Dp"]
    COLS16 = meta["COLS16"]
    ncolsB = meta["ncolsB"]
    f32 = mybir.dt.float32
    i16 = mybir.dt.int16
    Alu = mybir.AluOpType
    Act = mybir.ActivationFunctionType
    Axis = mybir.AxisListType
    nc = _mk_bass()

    bf16 = mybir.dt.bfloat16
    TBL_d = nc.dram_tensor("TBL", [NTAB, HID], f32, kind="ExternalInput")
    SELF_d = nc.dram_tensor("SELF", [P, T * HID], bf16, kind="ExternalInput")
    DINV_d = nc.dram_tensor("DINV", [P, T], f32, kind="ExternalInput")
    IDX_d = nc.dram_tensor("IDX16", [P, COLS16], i16, kind="ExternalInput")
    PIDX_d = nc.dram_tensor("PIDX", [P, 8 * Dp], i16, kind="ExternalInput")
    OUT_d = nc.dram_tensor("OUT", [HID, GPC], f32, kind="ExternalOutput")

    ptbl = nc.dram_tensor("ptbl", [1 + P * T, HID], f32)

    with tile.TileContext(nc, num_cores=C) as tc:
        with (
            tc.tile_pool(name="const", bufs=1) as const,
            tc.tile_pool(name="msg", bufs=3) as msgp,
            tc.tile_pool(name="zt", bufs=4) as ztp,
            tc.tile_pool(name="psum", bufs=2, space="PSUM") as psp,
        ):
            IDX_s = const.tile([P, COLS16], i16)
            if ncolsB:
                nc.sync.dma_start(IDX_s[:, 0:8 * ncolsB],
                                  IDX_d[:, 0:8 * ncolsB])
                nc.sync.dma_start(IDX_s[:, 8 * ncolsB:],
                                  IDX_d[:, 8 * ncolsB:])
            else:
                nc.sync.dma_start(IDX_s[:], IDX_d[:])
            DINV_s = const.tile([P, T], f32)
            nc.sync.dma_start(DINV_s[:], DINV_d[:])
            SELF_s = const.tile([P, T * HID], bf16)
            nc.sync.dma_start(SELF_s[:], SELF_d[:])
            PIDX_s = const.tile([P, 8 * Dp], i16)
            nc.sync.dma_start(PIDX_s[:], PIDX_d[:])
            ident = const.tile([P, P], f32)
            make_identity(nc, ident[:])
            nirow = const.tile([1, HID], f32)
            nc.vector.memset(nirow[:], float("-inf"))
            nc.sync.dma_start(ptbl[0:1, :], nirow[:])
            agg_strip = const.tile([P, T * HID], f32)
            out_strip = const.tile([P, T * HID], f32)
            Bbuf = const.tile([P, max(ncolsB, 1) * HID], f32)

            ptbl_rows = ptbl[1:1 + P * T, :].rearrange("(p t) f -> p t f",
                                                       p=P)

            def finish_tile(t, sl):
                nc.scalar.activation(
                    out_strip[:, t * HID:(t + 1) * HID], sl,
                    Act.Copy, scale=DINV_s[:, t:t + 1])

            def finish_group(t0, gsz):
                nc.sync.dma_start(
                    ptbl_rows[:, t0:t0 + gsz, :],
                    out_strip[:, t0 * HID:(t0 + gsz) * HID]
                        .rearrange("p (t f) -> p t f", f=HID))

            _emit_agg(nc, meta, TBL_d, IDX_s, Bbuf, agg_strip, SELF_s,
                      msgp, ztp, finish_tile, finish_group)

            pmsg = msgp.tile([P, Dp * HID], f32, tag="pmsg", name="pmsg")
            nc.gpsimd.dma_gather(
                out_ap=pmsg[:].rearrange("p (d f) -> p d f", f=HID),
                in_ap=ptbl[:],
                idxs_ap=PIDX_s[:],
                num_idxs=P * Dp,
                num_idxs_reg=P * Dp,
                elem_size=HID,
                single_packet=False,
            )
            poolA = ztp.tile([P, HID], f32, tag="poolA")
            nc.vector.tensor_reduce(
                out=poolA[:],
                in_=pmsg[:].rearrange("p (d f) -> p f d", f=HID),
                axis=Axis.X,
                op=Alu.max,
            )
            psP = psp.tile([HID, P], f32, tag="psT")
            nc.tensor.transpose(psP[:], poolA[:], ident[:])
            poolT = ztp.tile([HID, P], f32, tag="poolT")
            nc.scalar.activation(poolT[:], psP[:], Act.Copy)
            outsb = ztp.tile([HID, GPC], f32, tag="outsb")
            pt = poolT[:].rearrange("p (g two) -> p g two", two=2)
            nc.vector.tensor_tensor(out=outsb[:], in0=pt[:, :, 0],
                                    in1=pt[:, :, 1], op=Alu.max)
            nc.sync.dma_start(OUT_d[:], outsb[:])
    nc.compile()
    return nc


# --------------------------------------------------------------------------
# Entry point
# --------------------------------------------------------------------------

_RUN_KWARGS = {}
_EXEC_NS = []    # per-launch HW exec times when tracing enabled
_PROFILE = False


def _destripe(strip, T):
    """[128, T*HID] strip -> [Npc, HID] rows."""
    return strip.reshape(P, T, HID).transpose(1, 0, 2).reshape(T * P, HID)


def _mk_tables(strips, prep, meta):
    """Per-core launch-output strips -> per-core hot/cold-ordered tables."""
    T = meta["T"]
    NTAB = meta["NTAB"]
    t_full = np.zeros((N_NODES, HID), np.float32)
    for c in range(C):
        rows = _destripe(np.asarray(strips[c]).astype(np.float32), T)
        valid = prep["rowmaps"][c] >= 0
        t_full[prep["rowmaps"][c][valid]] = rows[valid]
    tabs = []
    for c in range(C):
        tab = np.zeros((NTAB, HID), np.float32)
        tab[1:1 + N_NODES] = t_full[prep["tabperms"][c]]
        tabs.append(tab)
    return tabs


def kernel(data, edge_index, batch, W1, b1, W2, b2, W3, b3):
    import ml_dtypes
    from concourse.bass_utils import run_bass_kernel_spmd

    bf16 = ml_dtypes.bfloat16
    data = np.asarray(data, dtype=np.float32)
    edge_index = np.asarray(edge_index, dtype=np.int32)
    batch_np = np.asarray(batch, dtype=np.int32)
    W1 = np.asarray(W1, dtype=np.float32)
    b1 = np.asarray(b1, dtype=np.float32)
    W2 = np.asarray(W2, dtype=np.float32)
    b2 = np.asarray(b2, dtype=np.float32)
    W3 = np.asarray(W3, dtype=np.float32)
    b3 = np.asarray(b3, dtype=np.float32)

    prep = _host_prep(edge_index, batch_np)
    meta = prep["meta"]
    T, Npc = meta["T"], meta["Npc"]
    s = prep["s"]

    cores = list(range(C))
    del _EXEC_NS[:]

    def run(nc, in_maps):
        if _PROFILE:
            from concourse.timeline_sim import TimelineSim
            _EXEC_NS.append(TimelineSim(nc, require_finite=False).simulate())
        res = run_bass_kernel_spmd(nc, in_maps, cores, **_RUN_KWARGS)
        if res.exec_time_ns is not None:
            _EXEC_NS.append(res.exec_time_ns)
        return res.results

    # strips of per-(p,t) values b/s^power for the bias folds
    def bias_fold(bvec, power):
        out = np.zeros((C, P, T * HID), np.float32)
        for c in range(C):
            pad = np.zeros(Npc, np.float64)
            valid = prep["rowmaps"][c] >= 0
            pad[valid] = s[prep["rowmaps"][c][valid]]
            cell = pad.reshape(T, P).T                    # [P, T]
            with np.errstate(divide="ignore"):
                f = np.where(cell > 0, 1.0 / (cell ** power), 0.0)
            out[c] = (f[:, :, None] * bvec[None, None, :]).reshape(P, T * HID)
        return out

    # ---- launch 1: T1 strips ----
    nc1 = _prog_tables(meta)
    xts = []
    for c in range(C):
        xt = np.zeros((IN_DIM, Npc), np.float32)
        valid = prep["rowmaps"][c] >= 0
        nodes = prep["rowmaps"][c][valid]
        xt[:, valid] = (data[nodes] * s[nodes][:, None]).T
        xts.append(xt.astype(bf16))
    W1b = W1.astype(bf16)
    r1 = run(nc1, [{"XT": xts[c], "W1": W1b} for c in range(C)])
    s1 = [np.asarray(r1[c]["OUTS"]).astype(np.float32) for c in range(C)]
    tabs1 = _mk_tables(s1, prep, meta)

    # ---- launch 2: layer 1 -> T2 strips ----
    nc2 = _prog_layer(meta, 1)
    fold1 = bias_fold(b1, 2)
    r2 = run(nc2, [{"TBL": tabs1[c],
                    "SELF": (s1[c] + fold1[c]).astype(bf16),
                    "DINV": np.ascontiguousarray(prep["dinvT"][1, c]),
                    "IDX16": np.ascontiguousarray(prep["idx16"][c])}
                   for c in range(C)])
    s2 = [np.asarray(r2[c]["OUTS"]).astype(np.float32) for c in range(C)]
    tabs2 = _mk_tables(s2, prep, meta)

    # ---- launch 3: layer 2 -> T3 strips ----
    nc3 = _prog_layer(meta, 2)
    W2b = W2.astype(bf16)
    W3b = W3.astype(bf16)
    B2col = b2.reshape(P, 1).astype(np.float32)
    r3 = run(nc3, [{"TBL": tabs2[c],
                    "SELF": s2[c].astype(bf16),
                    "DINV": np.ascontiguousarray(prep["dinvT"][0, c]),
                    "IDX16": np.ascontiguousarray(prep["idx16"][c]),
                    "W2": W2b, "W3": W3b, "B2": B2col}
                   for c in range(C)])
    s3 = [np.asarray(r3[c]["OUTS"]).astype(np.float32) for c in range(C)]
    tabs3 = _mk_tables(s3, prep, meta)

    # ---- launch 4: layer 3 + pool ----
    nc4 = _prog_final(meta)
    fold3 = bias_fold(b3, 1)
    r4 = run(nc4, [{"TBL": tabs3[c],
                    "SELF": (s3[c] + fold3[c]).astype(bf16),
                    "DINV": np.ascontiguousarray(prep["dinvT"][0, c]),
                    "IDX16": np.ascontiguousarray(prep["idx16"][c]),
                    "PIDX": np.ascontiguousarray(prep["pool16"][c])}
                   for c in range(C)])
    out = np.concatenate(
        [np.asarray(r4[c]["OUT"]).T for c in range(C)], axis=0
    )
    return out.astype(np.float32)


# revision 33
# speedup vs baseline: 1.0105x; 1.0105x over previous
"""Trainium2 Bass kernel for a 3-layer GCN encoder with global max pool.

Strategy (8 NeuronCores, SPMD, 4 launches):
  - Nodes partitioned graph-wise across cores (graph g -> core g//64).
  - GCN normalization factored out: with s = 1/sqrt(deg), table rows are
    pre-scaled (s*h), so aggregation is an unweighted gather+sum and all
    per-layer scaling collapses into per-(partition,tile) activation scales
    (relu(s*y) == s*relu(y) since s > 0).
  - Each layer's full node table lives in HBM per core.  Per-core tables are
    HOT/COLD ordered by that core's source-usage counts so the int16 gather
    index window [0, 32768) covers ~92% of edges; the cold window needs only
    ~60 padded slots total.  Destination nodes are sorted by (kB, kA) so the
    padded-CSR tiles are tight, with exact per-tile slot widths (no group-max
    padding).
  - dma_gather (padded CSR, one 256B descriptor per edge) + strided DVE
    tensor_reduce per 128-node tile.  One cold-window gather call per launch,
    hot-window gather calls per group of G tiles, heaviest groups first so
    the post-gather tail is short.  Everything after the per-tile reduce
    (self add, scale, bias, relu, the layer-2 W2/W3 matmul chain, output
    stores) is emitted per tile / per group so it pipelines under the
    remaining gather DMA.
  - Matmuls in bf16; PSUM<->SBUF moves, scaling, bias and relu are fused into
    single per-tile Activation-engine ops.  Launch outputs are contiguous
    strip stores; the host does de-striping/permutation between launches
    (the "AllGather").
  - Global max pool (launch 4): store the layer-3 strip per group, gather
    per-graph padded node lists (2 SBUF partitions per graph), reduce-max,
    PE transpose, pairwise max.
"""

import numpy as np

N_NODES = 50000
N_EDGES = 600000
IN_DIM = 128
HID = 64
N_GRAPHS = 512
C = 8           # cores
P = 128         # partitions
GPC = N_GRAPHS // C
SPLIT = 32768   # int16 index range per dma_gather call
G = 4           # tiles per hot-window gather call


def _pack_idx16(flat):
    """[num] int array -> wrapped [128, num//16] int16 (16-wrapped, 8x repl)."""
    num = flat.shape[0]
    assert num % 16 == 0
    arr = flat.reshape(num // 16, 16).T.astype(np.int16)   # [16, num//16]
    return np.tile(arr, (8, 1))                            # [128, num//16]


def _slots(vtile, vpart):
    """Running slot index within each (tile, partition) group."""
    order = np.lexsort((vpart, vtile))
    key = vtile[order] * P + vpart[order]
    newgrp = np.concatenate([[True], key[1:] != key[:-1]])
    gsp = np.nonzero(newgrp)[0]
    slot = np.arange(len(key)) - gsp[np.cumsum(newgrp) - 1]
    return order, slot


# --------------------------------------------------------------------------
# Host-side preprocessing: sharding, permutations, padded CSR index arrays.
# --------------------------------------------------------------------------

KBCAP = 3   # cold-edge-count band values: 0, 1, 2, and "3 or more"


def _host_prep(edge_index, batch):
    N = N_NODES
    src = np.asarray(edge_index[0], dtype=np.int64)
    dst = np.asarray(edge_index[1], dtype=np.int64)
    batch = np.asarray(batch, dtype=np.int64)

    indeg = np.bincount(dst, minlength=N)
    s = (1.0 / np.sqrt((indeg + 1).astype(np.float64))).astype(np.float64)

    core_of_node = batch // GPC
    dst_core = core_of_node[dst]
    NTAB = N + 2

    # pass 1: per-core hot/cold table order and per-dst hot/cold edge counts
    percore = []
    band_sizes = np.zeros((C, KBCAP + 1), np.int64)
    for c in range(C):
        m = dst_core == c
        s_c = src[m]
        d_c = dst[m]
        u = np.bincount(s_c, minlength=N)
        tabperm = np.argsort(-u, kind="stable")
        tabrow = np.empty(N, np.int64)
        tabrow[tabperm] = 1 + np.arange(N)
        erow = tabrow[s_c]
        lowm = erow < SPLIT
        nodes_c = np.nonzero(core_of_node == c)[0]
        kA = np.bincount(d_c[lowm], minlength=N)[nodes_c]
        kB = np.bincount(d_c[~lowm], minlength=N)[nodes_c]
        kb = np.minimum(kB, KBCAP)
        band_sizes[c] = np.bincount(kb, minlength=KBCAP + 1)
        percore.append((nodes_c, d_c, erow, lowm, kA, kB, kb, tabperm))

    # band-aligned tile layout: band b starts at the same tile on every core
    tiles_b = [int(-(-band_sizes[:, b].max() // P)) for b in range(KBCAP + 1)]
    bstart = np.concatenate([[0], np.cumsum(tiles_b)])
    T = int(bstart[-1])
    Npc = T * P

    rowmaps = []        # per core: node id at local row, -1 for pad cells
    tabperms = []       # per core: global node at table row 1+i
    e_cell = []         # per core: (erow, p, t, lowmask) for its in-edges
    for c in range(C):
        nodes_c, d_c, erow, lowm, kA, kB, kb, tabperm = percore[c]
        rowmap = np.full(Npc, -1, np.int64)
        row_of = np.full(N, -1, np.int64)
        for b in range(KBCAP + 1):
            sel = np.nonzero(kb == b)[0]
            order = sel[np.argsort(-kA[sel], kind="stable")]
            rows = bstart[b] * P + np.arange(len(order))
            rowmap[rows] = nodes_c[order]
            row_of[nodes_c[order]] = rows
        lp = row_of[d_c]
        rowmaps.append(rowmap)
        tabperms.append(tabperm)
        e_cell.append((erow, lp % P, lp // P, lowm))

    # per-tile slot widths (max over cores and partitions)
    DA_t = np.zeros(T, np.int64)
    DB_t = np.zeros(T, np.int64)
    for c in range(C):
        erow, vp, vt, lowm = e_cell[c]
        cntA = np.zeros((T, P), np.int64)
        np.add.at(cntA, (vt[lowm], vp[lowm]), 1)
        cntB = np.zeros((T, P), np.int64)
        np.add.at(cntB, (vt[~lowm], vp[~lowm]), 1)
        DA_t = np.maximum(DA_t, cntA.max(axis=1))
        DB_t = np.maximum(DB_t, cntB.max(axis=1))
    prefA = np.concatenate([[0], np.cumsum(DA_t)])
    prefB = np.concatenate([[0], np.cumsum(DB_t)])
    ncolsB = int(prefB[-1])
    padB = NTAB - 1 - SPLIT

    ngroups = -(-T // G)
    groups = []          # (col16_start, ncolsA, t0, gsz)
    col16 = 8 * ncolsB   # B block first
    for g in range(ngroups):
        t0 = g * G
        gsz = min(G, T - t0)
        ncA = int(prefA[t0 + gsz] - prefA[t0])
        groups.append((col16, ncA, t0, gsz))
        col16 += 8 * ncA
    COLS16 = col16

    idx16 = np.zeros((C, P, COLS16), np.int16)
    for c in range(C):
        erow, vp, vt, lowm = e_cell[c]
        # hot side
        er, p_, t_ = erow[lowm], vp[lowm], vt[lowm]
        order, slot = _slots(t_, p_)
        er, p_, t_ = er[order], p_[order], t_[order]
        g_ = t_ // G
        colg = (prefA[t_] - prefA[(t_ // G) * G]) + slot
        for g in range(ngroups):
            c16, ncA, t0, gsz = groups[g]
            if ncA == 0:
                continue
            flat = np.zeros(P * ncA, np.int64)
            m = g_ == g
            flat[colg[m] * P + p_[m]] = er[m]
            idx16[c][:, c16:c16 + 8 * ncA] = _pack_idx16(flat)
        # cold side
        er, p_, t_ = erow[~lowm] - SPLIT, vp[~lowm], vt[~lowm]
        order, slot = _slots(t_, p_)
        er, p_, t_ = er[order], p_[order], t_[order]
        if ncolsB:
            flat = np.full(P * ncolsB, padB, np.int64)
            flat[(prefB[t_] + slot) * P + p_] = er
            idx16[c][:, 0:8 * ncolsB] = _pack_idx16(flat)

    # per-(partition,tile) normalization scale strips: s, s^2
    dinvT = np.zeros((2, C, P, T), np.float64)
    for c in range(C):
        pad = np.zeros(Npc, np.float64)
        valid = rowmaps[c] >= 0
        pad[valid] = s[rowmaps[c][valid]]
        cell = pad.reshape(T, P).T                 # [P, T]
        dinvT[0, c] = cell
        dinvT[1, c] = cell ** 2
    dinvT = dinvT.astype(np.float32)

    # pooling CSR: graph local slot l -> partitions 2g, 2g+1 (alternating);
    # table row of node at local cell (p, t) in the stored strip is 1 + p*T + t
    Dp = 0
    pool_rows = []
    for c in range(C):
        loc = np.nonzero(rowmaps[c] >= 0)[0]
        gl = batch[rowmaps[c][loc]] % GPC
        order = np.lexsort((loc, gl))
        ogl, oloc = gl[order], loc[order]
        newg = np.concatenate([[True], ogl[1:] != ogl[:-1]])
        gsp = np.nonzero(newg)[0]
        gslot = np.arange(len(ogl)) - gsp[np.cumsum(newg) - 1]
        ppart = 2 * ogl + (gslot % 2)
        pslot = gslot // 2
        Dp = max(Dp, int(pslot.max()) + 1)
        pool_rows.append((ppart, pslot, 1 + (oloc % P) * T + (oloc // P)))
    pool16 = np.zeros((C, P, 8 * Dp), np.int16)
    for c in range(C):
        ppart, pslot, prow = pool_rows[c]
        flat = np.zeros(P * Dp, np.int64)          # pad -> row 0 (-inf row)
        flat[pslot * P + ppart] = prow
        pool16[c] = _pack_idx16(flat)

    meta = dict(T=T, Npc=Npc, NTAB=NTAB, COLS16=COLS16, ncolsB=ncolsB,
                prefA=prefA.tolist(), prefB=prefB.tolist(),
                DA_t=DA_t.tolist(), DB_t=DB_t.tolist(),
                groups=groups, Dp=Dp)
    return dict(idx16=idx16, pool16=pool16, dinvT=dinvT,
                rowmaps=rowmaps, tabperms=tabperms, s=s, meta=meta)


# --------------------------------------------------------------------------
# Bass programs (4 launches)
# --------------------------------------------------------------------------

def _mk_bass():
    import concourse.bacc as bacc
    return bacc.Bacc(None)


def _emit_agg(nc, meta, TBL_d, IDX_s, Bbuf, agg_strip, SELF_s, msgp, ztp,
              finish_tile, finish_group):
    """Gather + reduce + self-add per tile, heaviest groups first.

    finish_tile(t, sl): emit the per-tile tail (scale/relu/matmuls...)
    finish_group(t0, gsz): emit the per-group output store
    """
    import concourse.mybir as mybir
    f32 = mybir.dt.float32
    Alu = mybir.AluOpType
    Axis = mybir.AxisListType
    NTAB = meta["NTAB"]
    ncolsB = meta["ncolsB"]
    prefA, prefB = meta["prefA"], meta["prefB"]
    DA_t, DB_t = meta["DA_t"], meta["DB_t"]

    if ncolsB:
        nc.gpsimd.dma_gather(
            out_ap=Bbuf[:].rearrange("p (d f) -> p d f", f=HID),
            in_ap=TBL_d[SPLIT:NTAB, :],
            idxs_ap=IDX_s[:, 0:8 * ncolsB],
            num_idxs=P * ncolsB,
            num_idxs_reg=P * ncolsB,
            elem_size=HID,
            single_packet=False,
        )
    order = sorted(meta["groups"], key=lambda g: -g[1])   # heavy first
    for (c16, ncA, t0, gsz) in order:
        msg = None
        if ncA:
            msg = msgp.tile([P, ncA * HID], f32, tag="msg", name="msg")
            nc.gpsimd.dma_gather(
                out_ap=msg[:].rearrange("p (d f) -> p d f", f=HID),
                in_ap=TBL_d[0:SPLIT, :],
                idxs_ap=IDX_s[:, c16:c16 + 8 * ncA],
                num_idxs=P * ncA,
                num_idxs_reg=P * ncA,
                elem_size=HID,
                single_packet=False,
            )
        for t in range(t0, t0 + gsz):
            sl = agg_strip[:, t * HID:(t + 1) * HID]
            self_sl = SELF_s[:, t * HID:(t + 1) * HID]
            da, db = DA_t[t], DB_t[t]
            a0 = prefA[t] - prefA[t0]
            if da:
                nc.vector.tensor_reduce(
                    out=sl,
                    in_=msg[:, a0 * HID:(a0 + da) * HID]
                        .rearrange("p (d f) -> p f d", f=HID),
                    axis=Axis.X,
                    op=Alu.add,
                )
            if db:
                b0 = prefB[t]
                bap = Bbuf[:, b0 * HID:(b0 + db) * HID] \
                    .rearrange("p (d f) -> p f d", f=HID)
                if da:
                    tmp = ztp.tile([P, HID], f32, tag="btmp", name="btmp")
                    nc.vector.tensor_reduce(out=tmp[:], in_=bap,
                                            axis=Axis.X, op=Alu.add)
                    nc.vector.tensor_tensor(out=sl, in0=sl, in1=tmp[:],
                                            op=Alu.add)
                else:
                    nc.vector.tensor_reduce(out=sl, in_=bap,
                                            axis=Axis.X, op=Alu.add)
            if da or db:
                nc.vector.tensor_tensor(out=sl, in0=sl, in1=self_sl,
                                        op=Alu.add)
            else:
                nc.vector.tensor_copy(sl, self_sl)
            finish_tile(t, sl)
        finish_group(t0, gsz)


def _prog_tables(meta):
    """Launch 1: T1 strip = (s*X) @ W1 for this core's nodes (s folded into
    X on the host)."""
    import concourse.mybir as mybir
    import concourse.tile as tile

    T, Npc = meta["T"], meta["Npc"]
    f32 = mybir.dt.float32
    bf16 = mybir.dt.bfloat16
    Act = mybir.ActivationFunctionType
    nc = _mk_bass()

    XT_d = nc.dram_tensor("XT", [IN_DIM, Npc], bf16, kind="ExternalInput")
    W1_d = nc.dram_tensor("W1", [IN_DIM, HID], bf16, kind="ExternalInput")
    OUTS_d = nc.dram_tensor("OUTS", [P, T * HID], bf16,
                            kind="ExternalOutput")

    NCH = 4
    TC = -(-T // NCH)

    with tile.TileContext(nc, num_cores=C) as tc:
        with (
            tc.tile_pool(name="const", bufs=1) as const,
            tc.tile_pool(name="psum", bufs=4, space="PSUM") as psp,
        ):
            W1_s = const.tile([IN_DIM, HID], bf16)
            nc.sync.dma_start(W1_s[:], W1_d[:])
            chunks = []
            for ch in range(NCH):
                t0 = ch * TC
                tn = min(TC, T - t0)
                if tn <= 0:
                    break
                xt = const.tile([IN_DIM, tn * P], bf16, name=f"xt{ch}")
                nc.sync.dma_start(xt[:], XT_d[:, t0 * P:(t0 + tn) * P])
                chunks.append((t0, tn, xt))
            strip = const.tile([P, T * HID], bf16)
            for (t0, tn, xt) in chunks:
                for i in range(0, tn, 2):
                    t = t0 + i
                    n2 = min(2, tn - i)
                    ps = psp.tile([P, 2 * HID], f32, tag="ps", name="ps")
                    for j in range(n2):
                        nc.tensor.matmul(
                            ps[:, j * HID:(j + 1) * HID],
                            lhsT=xt[:, (i + j) * P:(i + j + 1) * P],
                            rhs=W1_s[:], start=True, stop=True)
                    # alternate PSUM->strip moves across Act and DVE so
                    # neither engine serializes the launch
                    if (i // 2) % 2 == 0:
                        nc.scalar.activation(
                            strip[:, t * HID:(t + n2) * HID],
                            ps[:, 0:n2 * HID], Act.Copy)
                    else:
                        nc.vector.tensor_copy(
                            strip[:, t * HID:(t + n2) * HID],
                            ps[:, 0:n2 * HID])
                nc.sync.dma_start(OUTS_d[:, t0 * HID:(t0 + tn) * HID],
                                  strip[:, t0 * HID:(t0 + tn) * HID])
    nc.compile()
    return nc


def _prog_layer(meta, layer):
    """Launches 2/3: aggregate TBL -> next table strip.

    layer=1: out = relu(s^2 * (Agg(T1) + SELFB)),  SELFB = self + b1/s^2
    layer=2: out = s * (h2 @ W3), h2 = relu((s*(Agg(T2)+self)) @ W2 + b2)
    """
    import concourse.mybir as mybir
    import concourse.tile as tile
    from concourse.masks import make_identity

    T, Npc, NTAB = meta["T"], meta["Npc"], meta["NTAB"]
    COLS16 = meta["COLS16"]
    ncolsB = meta["ncolsB"]
    f32 = mybir.dt.float32
    bf16 = mybir.dt.bfloat16
    i16 = mybir.dt.int16
    Alu = mybir.AluOpType
    Act = mybir.ActivationFunctionType
    nc = _mk_bass()

    TBL_d = nc.dram_tensor("TBL", [NTAB, HID], f32, kind="ExternalInput")
    SELF_d = nc.dram_tensor("SELF", [P, T * HID], bf16, kind="ExternalInput")
    DINV_d = nc.dram_tensor("DINV", [P, T], f32, kind="ExternalInput")
    IDX_d = nc.dram_tensor("IDX16", [P, COLS16], i16, kind="ExternalInput")
    OUTS_d = nc.dram_tensor("OUTS", [P, T * HID], bf16,
                            kind="ExternalOutput")
    if layer == 2:
        W2_d = nc.dram_tensor("W2", [HID, 2 * HID], bf16, kind="ExternalInput")
        W3_d = nc.dram_tensor("W3", [2 * HID, HID], bf16, kind="ExternalInput")
        B2_d = nc.dram_tensor("B2", [P, 1], f32, kind="ExternalInput")

    with tile.TileContext(nc, num_cores=C) as tc:
        with (
            tc.tile_pool(name="const", bufs=1) as const,
            tc.tile_pool(name="msg", bufs=3) as msgp,
            tc.tile_pool(name="zt", bufs=4) as ztp,
            tc.tile_pool(name="psum", bufs=2, space="PSUM") as psp,
        ):
            IDX_s = const.tile([P, COLS16], i16)
            if ncolsB:
                nc.sync.dma_start(IDX_s[:, 0:8 * ncolsB],
                                  IDX_d[:, 0:8 * ncolsB])
                nc.sync.dma_start(IDX_s[:, 8 * ncolsB:],
                                  IDX_d[:, 8 * ncolsB:])
            else:
                nc.sync.dma_start(IDX_s[:], IDX_d[:])
            DINV_s = const.tile([P, T], f32)
            nc.sync.dma_start(DINV_s[:], DINV_d[:])
            SELF_s = const.tile([P, T * HID], bf16)
            nc.sync.dma_start(SELF_s[:], SELF_d[:])
            if layer == 2:
                W2_s = const.tile([HID, 2 * HID], bf16)
                nc.sync.dma_start(W2_s[:], W2_d[:])
                W3_s = const.tile([2 * HID, HID], bf16)
                nc.sync.dma_start(W3_s[:], W3_d[:])
                B2_s = const.tile([P, 1], f32)
                nc.sync.dma_start(B2_s[:], B2_d[:])
                ident = const.tile([P, P], f32)
                make_identity(nc, ident[:])
                ident_bf = const.tile([P, P], bf16)
                nc.vector.tensor_copy(ident_bf[:], ident[:])
                aggbf = const.tile([P, T * HID], bf16)
            agg_strip = const.tile([P, T * HID], f32)
            out_strip = const.tile([P, T * HID], bf16)
            Bbuf = const.tile([P, max(ncolsB, 1) * HID], f32)

            if layer == 1:
                def finish_tile(t, sl):
                    nc.scalar.activation(
                        out_strip[:, t * HID:(t + 1) * HID], sl,
                        Act.Relu, scale=DINV_s[:, t:t + 1])
            else:
                def finish_tile(t, sl):
                    nc.scalar.activation(
                        aggbf[:, t * HID:(t + 1) * HID], sl,
                        Act.Copy, scale=DINV_s[:, t:t + 1])
                    psT = psp.tile([HID, P], bf16, tag="psT", name="psT")
                    nc.tensor.transpose(
                        psT[:], aggbf[:, t * HID:(t + 1) * HID], ident_bf[:])
                    zT = ztp.tile([HID, P], bf16, tag="zT", name="zT")
                    nc.scalar.activation(zT[:], psT[:], Act.Copy)
                    h2T = psp.tile([2 * HID, P], f32, tag="h2T", name="h2T")
                    nc.tensor.matmul(h2T[:], lhsT=W2_s[:], rhs=zT[:],
                                     start=True, stop=True)
                    h2Tbf = ztp.tile([2 * HID, P], bf16, tag="h2Tbf",
                                     name="h2Tbf")
                    nc.scalar.activation(h2Tbf[:], h2T[:], Act.Relu,
                                         bias=B2_s[:, 0:1])
                    ps3 = psp.tile([P, HID], f32, tag="ps3", name="ps3")
                    nc.tensor.matmul(ps3[:], lhsT=h2Tbf[:], rhs=W3_s[:],
                                     start=True, stop=True)
                    nc.vector.tensor_tensor(
                        out=out_strip[:, t * HID:(t + 1) * HID],
                        in0=ps3[:],
                        in1=DINV_s[:, t:t + 1].to_broadcast([P, HID]),
                        op=Alu.mult)

            def finish_group(t0, gsz):
                nc.sync.dma_start(
                    OUTS_d[:, t0 * HID:(t0 + gsz) * HID],
                    out_strip[:, t0 * HID:(t0 + gsz) * HID])

            _emit_agg(nc, meta, TBL_d, IDX_s, Bbuf, agg_strip, SELF_s,
                      msgp, ztp, finish_tile, finish_group)
    nc.compile()
    return nc


def _prog_final(meta):
    """Launch 4: layer-3 aggregation + bias, then global max pool."""
    import concourse.mybir as mybir
    import concourse.tile as tile
    from concourse.masks import make_identity

    T, Npc, NTAB, Dp = meta["T"], meta["Npc"], meta["NTAB"], meta["Dp"]
    COLS16 = meta["COLS16"]
    ncolsB = meta["ncolsB"]
    f32 = mybir.dt.float32
    i16 = mybir.dt.int16
    Alu = mybir.AluOpType
    Act = mybir.ActivationFunctionType
    Axis = mybir.AxisListType
    nc = _mk_bass()

    bf16 = mybir.dt.bfloat16
    TBL_d = nc.dram_tensor("TBL", [NTAB, HID], f32, kind="ExternalInput")
    SELF_d = nc.dram_tensor("SELF", [P, T * HID], bf16, kind="ExternalInput")
    DINV_d = nc.dram_tensor("DINV", [P, T], f32, kind="ExternalInput")
    IDX_d = nc.dram_tensor("IDX16", [P, COLS16], i16, kind="ExternalInput")
    PIDX_d = nc.dram_tensor("PIDX", [P, 8 * Dp], i16, kind="ExternalInput")
    OUT_d = nc.dram_tensor("OUT", [HID, GPC], f32, kind="ExternalOutput")

    ptbl = nc.dram_tensor("ptbl", [1 + P * T, HID], f32)

    with tile.TileContext(nc, num_cores=C) as tc:
        with (
            tc.tile_pool(name="const", bufs=1) as const,
            tc.tile_pool(name="msg", bufs=3) as msgp,
            tc.tile_pool(name="zt", bufs=4) as ztp,
            tc.tile_pool(name="psum", bufs=2, space="PSUM") as psp,
        ):
            IDX_s = const.tile([P, COLS16], i16)
            if ncolsB:
                nc.sync.dma_start(IDX_s[:, 0:8 * ncolsB],
                                  IDX_d[:, 0:8 * ncolsB])
                nc.sync.dma_start(IDX_s[:, 8 * ncolsB:],
                                  IDX_d[:, 8 * ncolsB:])
            else:
                nc.sync.dma_start(IDX_s[:], IDX_d[:])
            DINV_s = const.tile([P, T], f32)
            nc.sync.dma_start(DINV_s[:], DINV_d[:])
            SELF_s = const.tile([P, T * HID], bf16)
            nc.sync.dma_start(SELF_s[:], SELF_d[:])
            PIDX_s = const.tile([P, 8 * Dp], i16)
            nc.sync.dma_start(PIDX_s[:], PIDX_d[:])
            ident = const.tile([P, P], f32)
            make_identity(nc, ident[:])
            nirow = const.tile([1, HID], f32)
            nc.vector.memset(nirow[:], float("-inf"))
            nc.sync.dma_start(ptbl[0:1, :], nirow[:])
            agg_strip = const.tile([P, T * HID], f32)
            out_strip = const.tile([P, T * HID], f32)
            Bbuf = const.tile([P, max(ncolsB, 1) * HID], f32)

            ptbl_rows = ptbl[1:1 + P * T, :].rearrange("(p t) f -> p t f",
                                                       p=P)

            def finish_tile(t, sl):
                nc.scalar.activation(
                    out_strip[:, t * HID:(t + 1) * HID], sl,
                    Act.Copy, scale=DINV_s[:, t:t + 1])

            def finish_group(t0, gsz):
                nc.sync.dma_start(
                    ptbl_rows[:, t0:t0 + gsz, :],
                    out_strip[:, t0 * HID:(t0 + gsz) * HID]
                        .rearrange("p (t f) -> p t f", f=HID))

            _emit_agg(nc, meta, TBL_d, IDX_s, Bbuf, agg_strip, SELF_s,
                      msgp, ztp, finish_tile, finish_group)

            # two half-gathers: the second half's transfer overlaps the
            # first half's reduce
            Dp1 = (Dp + 1) // 2
            halves = []
            for (d0, dn) in ((0, Dp1), (Dp1, Dp - Dp1)):
                pmsg = msgp.tile([P, dn * HID], f32, tag="pmsg",
                                 name=f"pmsg{d0}")
                nc.gpsimd.dma_gather(
                    out_ap=pmsg[:].rearrange("p (d f) -> p d f", f=HID),
                    in_ap=ptbl[:],
                    idxs_ap=PIDX_s[:, 8 * d0:8 * (d0 + dn)],
                    num_idxs=P * dn,
                    num_idxs_reg=P * dn,
                    elem_size=HID,
                    single_packet=False,
                )
                pa = ztp.tile([P, HID], f32, tag=f"poolA{d0}",
                              name=f"poolA{d0}")
                nc.vector.tensor_reduce(
                    out=pa[:],
                    in_=pmsg[:].rearrange("p (d f) -> p f d", f=HID),
                    axis=Axis.X,
                    op=Alu.max,
                )
                halves.append(pa)
            poolA = ztp.tile([P, HID], f32, tag="poolA")
            nc.vector.tensor_tensor(out=poolA[:], in0=halves[0][:],
                                    in1=halves[1][:], op=Alu.max)
            psP = psp.tile([HID, P], f32, tag="psT")
            nc.tensor.transpose(psP[:], poolA[:], ident[:])
            poolT = ztp.tile([HID, P], f32, tag="poolT")
            nc.scalar.activation(poolT[:], psP[:], Act.Copy)
            outsb = ztp.tile([HID, GPC], f32, tag="outsb")
            pt = poolT[:].rearrange("p (g two) -> p g two", two=2)
            nc.vector.tensor_tensor(out=outsb[:], in0=pt[:, :, 0],
                                    in1=pt[:, :, 1], op=Alu.max)
            nc.sync.dma_start(OUT_d[:], outsb[:])
    nc.compile()
    return nc


# --------------------------------------------------------------------------
# Entry point
# --------------------------------------------------------------------------

_RUN_KWARGS = {}
_EXEC_NS = []    # per-launch HW exec times when tracing enabled
_PROFILE = False


def _destripe(strip, T):
    """[128, T*HID] strip -> [Npc, HID] rows."""
    return strip.reshape(P, T, HID).transpose(1, 0, 2).reshape(T * P, HID)


def _mk_tables(strips, prep, meta):
    """Per-core launch-output strips -> per-core hot/cold-ordered tables."""
    T = meta["T"]
    NTAB = meta["NTAB"]
    t_full = np.zeros((N_NODES, HID), np.float32)
    for c in range(C):
        rows = _destripe(np.asarray(strips[c]).astype(np.float32), T)
        valid = prep["rowmaps"][c] >= 0
        t_full[prep["rowmaps"][c][valid]] = rows[valid]
    tabs = []
    for c in range(C):
        tab = np.zeros((NTAB, HID), np.float32)
        tab[1:1 + N_NODES] = t_full[prep["tabperms"][c]]
        tabs.append(tab)
    return tabs


def kernel(data, edge_index, batch, W1, b1, W2, b2, W3, b3):
    import ml_dtypes
    from concourse.bass_utils import run_bass_kernel_spmd

    bf16 = ml_dtypes.bfloat16
    data = np.asarray(data, dtype=np.float32)
    edge_index = np.asarray(edge_index, dtype=np.int32)
    batch_np = np.asarray(batch, dtype=np.int32)
    W1 = np.asarray(W1, dtype=np.float32)
    b1 = np.asarray(b1, dtype=np.float32)
    W2 = np.asarray(W2, dtype=np.float32)
    b2 = np.asarray(b2, dtype=np.float32)
    W3 = np.asarray(W3, dtype=np.float32)
    b3 = np.asarray(b3, dtype=np.float32)

    prep = _host_prep(edge_index, batch_np)
    meta = prep["meta"]
    T, Npc = meta["T"], meta["Npc"]
    s = prep["s"]

    cores = list(range(C))
    del _EXEC_NS[:]

    def run(nc, in_maps):
        if _PROFILE:
            from concourse.timeline_sim import TimelineSim
            _EXEC_NS.append(TimelineSim(nc, require_finite=False).simulate())
        res = run_bass_kernel_spmd(nc, in_maps, cores, **_RUN_KWARGS)
        if res.exec_time_ns is not None:
            _EXEC_NS.append(res.exec_time_ns)
        return res.results

    # strips of per-(p,t) values b/s^power for the bias folds
    def bias_fold(bvec, power):
        out = np.zeros((C, P, T * HID), np.float32)
        for c in range(C):
            pad = np.zeros(Npc, np.float64)
            valid = prep["rowmaps"][c] >= 0
            pad[valid] = s[prep["rowmaps"][c][valid]]
            cell = pad.reshape(T, P).T                    # [P, T]
            with np.errstate(divide="ignore"):
                f = np.where(cell > 0, 1.0 / (cell ** power), 0.0)
            out[c] = (f[:, :, None] * bvec[None, None, :]).reshape(P, T * HID)
        return out

    # ---- launch 1: T1 strips ----
    nc1 = _prog_tables(meta)
    xts = []
    for c in range(C):
        xt = np.zeros((IN_DIM, Npc), np.float32)
        valid = prep["rowmaps"][c] >= 0
        nodes = prep["rowmaps"][c][valid]
        xt[:, valid] = (data[nodes] * s[nodes][:, None]).T
        xts.append(xt.astype(bf16))
    W1b = W1.astype(bf16)
    r1 = run(nc1, [{"XT": xts[c], "W1": W1b} for c in range(C)])
    s1 = [np.asarray(r1[c]["OUTS"]).astype(np.float32) for c in range(C)]
    tabs1 = _mk_tables(s1, prep, meta)

    # ---- launch 2: layer 1 -> T2 strips ----
    nc2 = _prog_layer(meta, 1)
    fold1 = bias_fold(b1, 2)
    r2 = run(nc2, [{"TBL": tabs1[c],
                    "SELF": (s1[c] + fold1[c]).astype(bf16),
                    "DINV": np.ascontiguousarray(prep["dinvT"][1, c]),
                    "IDX16": np.ascontiguousarray(prep["idx16"][c])}
                   for c in range(C)])
    s2 = [np.asarray(r2[c]["OUTS"]).astype(np.float32) for c in range(C)]
    tabs2 = _mk_tables(s2, prep, meta)

    # ---- launch 3: layer 2 -> T3 strips ----
    nc3 = _prog_layer(meta, 2)
    W2b = W2.astype(bf16)
    W3b = W3.astype(bf16)
    B2col = b2.reshape(P, 1).astype(np.float32)
    r3 = run(nc3, [{"TBL": tabs2[c],
                    "SELF": s2[c].astype(bf16),
                    "DINV": np.ascontiguousarray(prep["dinvT"][0, c]),
                    "IDX16": np.ascontiguousarray(prep["idx16"][c]),
                    "W2": W2b, "W3": W3b, "B2": B2col}
                   for c in range(C)])
    s3 = [np.asarray(r3[c]["OUTS"]).astype(np.float32) for c in range(C)]
    tabs3 = _mk_tables(s3, prep, meta)

    # ---- launch 4: layer 3 + pool ----
    nc4 = _prog_final(meta)
    fold3 = bias_fold(b3, 1)
    r4 = run(nc4, [{"TBL": tabs3[c],
                    "SELF": (s3[c] + fold3[c]).astype(bf16),
                    "DINV": np.ascontiguousarray(prep["dinvT"][0, c]),
                    "IDX16": np.ascontiguousarray(prep["idx16"][c]),
                    "PIDX": np.ascontiguousarray(prep["pool16"][c])}
                   for c in range(C)])
    out = np.concatenate(
        [np.asarray(r4[c]["OUT"]).T for c in range(C)], axis=0
    )
    return out.astype(np.float32)


# revision 34
# speedup vs baseline: 1.0111x; 1.0006x over previous
"""Trainium2 Bass kernel for a 3-layer GCN encoder with global max pool.

Strategy (8 NeuronCores, SPMD, 4 launches):
  - Nodes partitioned graph-wise across cores (graph g -> core g//64).
  - GCN normalization factored out: with s = 1/sqrt(deg), table rows are
    pre-scaled (s*h), so aggregation is an unweighted gather+sum and all
    per-layer scaling collapses into per-(partition,tile) activation scales
    (relu(s*y) == s*relu(y) since s > 0).
  - Each layer's full node table lives in HBM per core.  Per-core tables are
    HOT/COLD ordered by that core's source-usage counts so the int16 gather
    index window [0, 32768) covers ~92% of edges; the cold window needs only
    ~60 padded slots total.  Destination nodes are sorted by (kB, kA) so the
    padded-CSR tiles are tight, with exact per-tile slot widths (no group-max
    padding).
  - dma_gather (padded CSR, one 256B descriptor per edge) + strided DVE
    tensor_reduce per 128-node tile.  One cold-window gather call per launch,
    hot-window gather calls per group of G tiles, heaviest groups first so
    the post-gather tail is short.  Everything after the per-tile reduce
    (self add, scale, bias, relu, the layer-2 W2/W3 matmul chain, output
    stores) is emitted per tile / per group so it pipelines under the
    remaining gather DMA.
  - Matmuls in bf16; PSUM<->SBUF moves, scaling, bias and relu are fused into
    single per-tile Activation-engine ops.  Launch outputs are contiguous
    strip stores; the host does de-striping/permutation between launches
    (the "AllGather").
  - Global max pool (launch 4): store the layer-3 strip per group, gather
    per-graph padded node lists (2 SBUF partitions per graph), reduce-max,
    PE transpose, pairwise max.
"""

import numpy as np

N_NODES = 50000
N_EDGES = 600000
IN_DIM = 128
HID = 64
N_GRAPHS = 512
C = 8           # cores
P = 128         # partitions
GPC = N_GRAPHS // C
SPLIT = 32768   # int16 index range per dma_gather call
G = 4           # tiles per hot-window gather call


def _pack_idx16(flat):
    """[num] int array -> wrapped [128, num//16] int16 (16-wrapped, 8x repl)."""
    num = flat.shape[0]
    assert num % 16 == 0
    arr = flat.reshape(num // 16, 16).T.astype(np.int16)   # [16, num//16]
    return np.tile(arr, (8, 1))                            # [128, num//16]


def _slots(vtile, vpart):
    """Running slot index within each (tile, partition) group."""
    order = np.lexsort((vpart, vtile))
    key = vtile[order] * P + vpart[order]
    newgrp = np.concatenate([[True], key[1:] != key[:-1]])
    gsp = np.nonzero(newgrp)[0]
    slot = np.arange(len(key)) - gsp[np.cumsum(newgrp) - 1]
    return order, slot


# --------------------------------------------------------------------------
# Host-side preprocessing: sharding, permutations, padded CSR index arrays.
# --------------------------------------------------------------------------

KBCAP = 3   # cold-edge-count band values: 0, 1, 2, and "3 or more"


def _host_prep(edge_index, batch):
    N = N_NODES
    src = np.asarray(edge_index[0], dtype=np.int64)
    dst = np.asarray(edge_index[1], dtype=np.int64)
    batch = np.asarray(batch, dtype=np.int64)

    indeg = np.bincount(dst, minlength=N)
    s = (1.0 / np.sqrt((indeg + 1).astype(np.float64))).astype(np.float64)

    core_of_node = batch // GPC
    dst_core = core_of_node[dst]
    NTAB = N + 2

    # pass 1: per-core hot/cold table order and per-dst hot/cold edge counts
    percore = []
    band_sizes = np.zeros((C, KBCAP + 1), np.int64)
    for c in range(C):
        m = dst_core == c
        s_c = src[m]
        d_c = dst[m]
        u = np.bincount(s_c, minlength=N)
        tabperm = np.argsort(-u, kind="stable")
        tabrow = np.empty(N, np.int64)
        tabrow[tabperm] = 1 + np.arange(N)
        erow = tabrow[s_c]
        lowm = erow < SPLIT
        nodes_c = np.nonzero(core_of_node == c)[0]
        kA = np.bincount(d_c[lowm], minlength=N)[nodes_c]
        kB = np.bincount(d_c[~lowm], minlength=N)[nodes_c]
        kb = np.minimum(kB, KBCAP)
        band_sizes[c] = np.bincount(kb, minlength=KBCAP + 1)
        percore.append((nodes_c, d_c, erow, lowm, kA, kB, kb, tabperm))

    # band-aligned tile layout: band b starts at the same tile on every core
    tiles_b = [int(-(-band_sizes[:, b].max() // P)) for b in range(KBCAP + 1)]
    bstart = np.concatenate([[0], np.cumsum(tiles_b)])
    T = int(bstart[-1])
    Npc = T * P

    rowmaps = []        # per core: node id at local row, -1 for pad cells
    tabperms = []       # per core: global node at table row 1+i
    e_cell = []         # per core: (erow, p, t, lowmask) for its in-edges
    for c in range(C):
        nodes_c, d_c, erow, lowm, kA, kB, kb, tabperm = percore[c]
        rowmap = np.full(Npc, -1, np.int64)
        row_of = np.full(N, -1, np.int64)
        for b in range(KBCAP + 1):
            sel = np.nonzero(kb == b)[0]
            order = sel[np.argsort(-kA[sel], kind="stable")]
            rows = bstart[b] * P + np.arange(len(order))
            rowmap[rows] = nodes_c[order]
            row_of[nodes_c[order]] = rows
        lp = row_of[d_c]
        rowmaps.append(rowmap)
        tabperms.append(tabperm)
        e_cell.append((erow, lp % P, lp // P, lowm))

    # per-tile slot widths (max over cores and partitions)
    DA_t = np.zeros(T, np.int64)
    DB_t = np.zeros(T, np.int64)
    for c in range(C):
        erow, vp, vt, lowm = e_cell[c]
        cntA = np.zeros((T, P), np.int64)
        np.add.at(cntA, (vt[lowm], vp[lowm]), 1)
        cntB = np.zeros((T, P), np.int64)
        np.add.at(cntB, (vt[~lowm], vp[~lowm]), 1)
        DA_t = np.maximum(DA_t, cntA.max(axis=1))
        DB_t = np.maximum(DB_t, cntB.max(axis=1))
    prefA = np.concatenate([[0], np.cumsum(DA_t)])
    prefB = np.concatenate([[0], np.cumsum(DB_t)])
    ncolsB = int(prefB[-1])
    padB = NTAB - 1 - SPLIT

    ngroups = -(-T // G)
    groups = []          # (col16_start, ncolsA, t0, gsz)
    col16 = 8 * ncolsB   # B block first
    for g in range(ngroups):
        t0 = g * G
        gsz = min(G, T - t0)
        ncA = int(prefA[t0 + gsz] - prefA[t0])
        groups.append((col16, ncA, t0, gsz))
        col16 += 8 * ncA
    COLS16 = col16

    idx16 = np.zeros((C, P, COLS16), np.int16)
    for c in range(C):
        erow, vp, vt, lowm = e_cell[c]
        # hot side
        er, p_, t_ = erow[lowm], vp[lowm], vt[lowm]
        order, slot = _slots(t_, p_)
        er, p_, t_ = er[order], p_[order], t_[order]
        g_ = t_ // G
        colg = (prefA[t_] - prefA[(t_ // G) * G]) + slot
        for g in range(ngroups):
            c16, ncA, t0, gsz = groups[g]
            if ncA == 0:
                continue
            flat = np.zeros(P * ncA, np.int64)
            m = g_ == g
            flat[colg[m] * P + p_[m]] = er[m]
            idx16[c][:, c16:c16 + 8 * ncA] = _pack_idx16(flat)
        # cold side
        er, p_, t_ = erow[~lowm] - SPLIT, vp[~lowm], vt[~lowm]
        order, slot = _slots(t_, p_)
        er, p_, t_ = er[order], p_[order], t_[order]
        if ncolsB:
            flat = np.full(P * ncolsB, padB, np.int64)
            flat[(prefB[t_] + slot) * P + p_] = er
            idx16[c][:, 0:8 * ncolsB] = _pack_idx16(flat)

    # per-(partition,tile) normalization scale strips: s, s^2
    dinvT = np.zeros((2, C, P, T), np.float64)
    for c in range(C):
        pad = np.zeros(Npc, np.float64)
        valid = rowmaps[c] >= 0
        pad[valid] = s[rowmaps[c][valid]]
        cell = pad.reshape(T, P).T                 # [P, T]
        dinvT[0, c] = cell
        dinvT[1, c] = cell ** 2
    dinvT = dinvT.astype(np.float32)

    # pooling CSR: graph local slot l -> partitions 2g, 2g+1 (alternating);
    # table row of node at local cell (p, t) in the stored strip is 1 + p*T + t
    Dp = 0
    pool_rows = []
    for c in range(C):
        loc = np.nonzero(rowmaps[c] >= 0)[0]
        gl = batch[rowmaps[c][loc]] % GPC
        order = np.lexsort((loc, gl))
        ogl, oloc = gl[order], loc[order]
        newg = np.concatenate([[True], ogl[1:] != ogl[:-1]])
        gsp = np.nonzero(newg)[0]
        gslot = np.arange(len(ogl)) - gsp[np.cumsum(newg) - 1]
        ppart = 2 * ogl + (gslot % 2)
        pslot = gslot // 2
        Dp = max(Dp, int(pslot.max()) + 1)
        pool_rows.append((ppart, pslot, 1 + (oloc % P) * T + (oloc // P)))
    pool16 = np.zeros((C, P, 8 * Dp), np.int16)
    for c in range(C):
        ppart, pslot, prow = pool_rows[c]
        flat = np.zeros(P * Dp, np.int64)          # pad -> row 0 (-inf row)
        flat[pslot * P + ppart] = prow
        pool16[c] = _pack_idx16(flat)

    meta = dict(T=T, Npc=Npc, NTAB=NTAB, COLS16=COLS16, ncolsB=ncolsB,
                prefA=prefA.tolist(), prefB=prefB.tolist(),
                DA_t=DA_t.tolist(), DB_t=DB_t.tolist(),
                groups=groups, Dp=Dp)
    return dict(idx16=idx16, pool16=pool16, dinvT=dinvT,
                rowmaps=rowmaps, tabperms=tabperms, s=s, meta=meta)


# --------------------------------------------------------------------------
# Bass programs (4 launches)
# --------------------------------------------------------------------------

def _mk_bass():
    import concourse.bacc as bacc
    return bacc.Bacc(None)


def _emit_agg(nc, meta, TBL_d, IDX_s, Bbuf, agg_strip, SELF_s, msgp, ztp,
              finish_tile, finish_group):
    """Gather + reduce + self-add per tile, heaviest groups first.

    finish_tile(t, sl): emit the per-tile tail (scale/relu/matmuls...)
    finish_group(t0, gsz): emit the per-group output store
    """
    import concourse.mybir as mybir
    f32 = mybir.dt.float32
    Alu = mybir.AluOpType
    Axis = mybir.AxisListType
    NTAB = meta["NTAB"]
    ncolsB = meta["ncolsB"]
    prefA, prefB = meta["prefA"], meta["prefB"]
    DA_t, DB_t = meta["DA_t"], meta["DB_t"]

    if ncolsB:
        nc.gpsimd.dma_gather(
            out_ap=Bbuf[:].rearrange("p (d f) -> p d f", f=HID),
            in_ap=TBL_d[SPLIT:NTAB, :],
            idxs_ap=IDX_s[:, 0:8 * ncolsB],
            num_idxs=P * ncolsB,
            num_idxs_reg=P * ncolsB,
            elem_size=HID,
            single_packet=False,
        )
    order = sorted(meta["groups"], key=lambda g: -g[1])   # heavy first
    for (c16, ncA, t0, gsz) in order:
        msg = None
        if ncA:
            msg = msgp.tile([P, ncA * HID], f32, tag="msg", name="msg")
            nc.gpsimd.dma_gather(
                out_ap=msg[:].rearrange("p (d f) -> p d f", f=HID),
                in_ap=TBL_d[0:SPLIT, :],
                idxs_ap=IDX_s[:, c16:c16 + 8 * ncA],
                num_idxs=P * ncA,
                num_idxs_reg=P * ncA,
                elem_size=HID,
                single_packet=False,
            )
        for t in range(t0, t0 + gsz):
            sl = agg_strip[:, t * HID:(t + 1) * HID]
            self_sl = SELF_s[:, t * HID:(t + 1) * HID]
            da, db = DA_t[t], DB_t[t]
            a0 = prefA[t] - prefA[t0]
            if da:
                nc.vector.tensor_reduce(
                    out=sl,
                    in_=msg[:, a0 * HID:(a0 + da) * HID]
                        .rearrange("p (d f) -> p f d", f=HID),
                    axis=Axis.X,
                    op=Alu.add,
                )
            if db:
                b0 = prefB[t]
                bap = Bbuf[:, b0 * HID:(b0 + db) * HID] \
                    .rearrange("p (d f) -> p f d", f=HID)
                if da:
                    tmp = ztp.tile([P, HID], f32, tag="btmp", name="btmp")
                    nc.vector.tensor_reduce(out=tmp[:], in_=bap,
                                            axis=Axis.X, op=Alu.add)
                    nc.vector.tensor_tensor(out=sl, in0=sl, in1=tmp[:],
                                            op=Alu.add)
                else:
                    nc.vector.tensor_reduce(out=sl, in_=bap,
                                            axis=Axis.X, op=Alu.add)
            if da or db:
                nc.vector.tensor_tensor(out=sl, in0=sl, in1=self_sl,
                                        op=Alu.add)
            else:
                nc.vector.tensor_copy(sl, self_sl)
            finish_tile(t, sl)
        finish_group(t0, gsz)


def _prog_tables(meta):
    """Launch 1: T1 strip = (s*X) @ W1 for this core's nodes (s folded into
    X on the host)."""
    import concourse.mybir as mybir
    import concourse.tile as tile

    T, Npc = meta["T"], meta["Npc"]
    f32 = mybir.dt.float32
    bf16 = mybir.dt.bfloat16
    Act = mybir.ActivationFunctionType
    nc = _mk_bass()

    XT_d = nc.dram_tensor("XT", [IN_DIM, Npc], bf16, kind="ExternalInput")
    W1_d = nc.dram_tensor("W1", [IN_DIM, HID], bf16, kind="ExternalInput")
    OUTS_d = nc.dram_tensor("OUTS", [P, T * HID], bf16,
                            kind="ExternalOutput")

    NCH = 4
    TC = -(-T // NCH)

    with tile.TileContext(nc, num_cores=C) as tc:
        with (
            tc.tile_pool(name="const", bufs=1) as const,
            tc.tile_pool(name="psum", bufs=4, space="PSUM") as psp,
        ):
            W1_s = const.tile([IN_DIM, HID], bf16)
            nc.sync.dma_start(W1_s[:], W1_d[:])
            chunks = []
            for ch in range(NCH):
                t0 = ch * TC
                tn = min(TC, T - t0)
                if tn <= 0:
                    break
                xt = const.tile([IN_DIM, tn * P], bf16, name=f"xt{ch}")
                nc.sync.dma_start(xt[:], XT_d[:, t0 * P:(t0 + tn) * P])
                chunks.append((t0, tn, xt))
            strip = const.tile([P, T * HID], bf16)
            for (t0, tn, xt) in chunks:
                for i in range(0, tn, 2):
                    t = t0 + i
                    n2 = min(2, tn - i)
                    ps = psp.tile([P, 2 * HID], f32, tag="ps", name="ps")
                    for j in range(n2):
                        nc.tensor.matmul(
                            ps[:, j * HID:(j + 1) * HID],
                            lhsT=xt[:, (i + j) * P:(i + j + 1) * P],
                            rhs=W1_s[:], start=True, stop=True)
                    # alternate PSUM->strip moves across Act and DVE so
                    # neither engine serializes the launch
                    if (i // 2) % 2 == 0:
                        nc.scalar.activation(
                            strip[:, t * HID:(t + n2) * HID],
                            ps[:, 0:n2 * HID], Act.Copy)
                    else:
                        nc.vector.tensor_copy(
                            strip[:, t * HID:(t + n2) * HID],
                            ps[:, 0:n2 * HID])
                nc.sync.dma_start(OUTS_d[:, t0 * HID:(t0 + tn) * HID],
                                  strip[:, t0 * HID:(t0 + tn) * HID])
    nc.compile()
    return nc


def _prog_layer(meta, layer):
    """Launches 2/3: aggregate TBL -> next table strip.

    layer=1: out = relu(s^2 * (Agg(T1) + SELFB)),  SELFB = self + b1/s^2
    layer=2: out = s * (h2 @ W3), h2 = relu((s*(Agg(T2)+self)) @ W2 + b2)
    """
    import concourse.mybir as mybir
    import concourse.tile as tile
    from concourse.masks import make_identity

    T, Npc, NTAB = meta["T"], meta["Npc"], meta["NTAB"]
    COLS16 = meta["COLS16"]
    ncolsB = meta["ncolsB"]
    f32 = mybir.dt.float32
    bf16 = mybir.dt.bfloat16
    i16 = mybir.dt.int16
    Alu = mybir.AluOpType
    Act = mybir.ActivationFunctionType
    nc = _mk_bass()

    TBL_d = nc.dram_tensor("TBL", [NTAB, HID], f32, kind="ExternalInput")
    SELF_d = nc.dram_tensor("SELF", [P, T * HID], bf16, kind="ExternalInput")
    DINV_d = nc.dram_tensor("DINV", [P, T], f32, kind="ExternalInput")
    IDX_d = nc.dram_tensor("IDX16", [P, COLS16], i16, kind="ExternalInput")
    OUTS_d = nc.dram_tensor("OUTS", [P, T * HID], bf16,
                            kind="ExternalOutput")
    if layer == 2:
        W2_d = nc.dram_tensor("W2", [HID, 2 * HID], bf16, kind="ExternalInput")
        W3_d = nc.dram_tensor("W3", [2 * HID, HID], bf16, kind="ExternalInput")
        B2_d = nc.dram_tensor("B2", [P, 1], f32, kind="ExternalInput")

    with tile.TileContext(nc, num_cores=C) as tc:
        with (
            tc.tile_pool(name="const", bufs=1) as const,
            tc.tile_pool(name="msg", bufs=4) as msgp,
            tc.tile_pool(name="zt", bufs=6) as ztp,
            tc.tile_pool(name="psum", bufs=2, space="PSUM") as psp,
        ):
            IDX_s = const.tile([P, COLS16], i16)
            if ncolsB:
                nc.sync.dma_start(IDX_s[:, 0:8 * ncolsB],
                                  IDX_d[:, 0:8 * ncolsB])
                nc.sync.dma_start(IDX_s[:, 8 * ncolsB:],
                                  IDX_d[:, 8 * ncolsB:])
            else:
                nc.sync.dma_start(IDX_s[:], IDX_d[:])
            DINV_s = const.tile([P, T], f32)
            nc.sync.dma_start(DINV_s[:], DINV_d[:])
            SELF_s = const.tile([P, T * HID], bf16)
            nc.sync.dma_start(SELF_s[:], SELF_d[:])
            if layer == 2:
                W2_s = const.tile([HID, 2 * HID], bf16)
                nc.sync.dma_start(W2_s[:], W2_d[:])
                W3_s = const.tile([2 * HID, HID], bf16)
                nc.sync.dma_start(W3_s[:], W3_d[:])
                B2_s = const.tile([P, 1], f32)
                nc.sync.dma_start(B2_s[:], B2_d[:])
                ident = const.tile([P, P], f32)
                make_identity(nc, ident[:])
                ident_bf = const.tile([P, P], bf16)
                nc.vector.tensor_copy(ident_bf[:], ident[:])
                aggbf = const.tile([P, T * HID], bf16)
            agg_strip = const.tile([P, T * HID], f32)
            out_strip = const.tile([P, T * HID], bf16)
            Bbuf = const.tile([P, max(ncolsB, 1) * HID], f32)

            if layer == 1:
                def finish_tile(t, sl):
                    nc.scalar.activation(
                        out_strip[:, t * HID:(t + 1) * HID], sl,
                        Act.Relu, scale=DINV_s[:, t:t + 1])
            else:
                def finish_tile(t, sl):
                    nc.scalar.activation(
                        aggbf[:, t * HID:(t + 1) * HID], sl,
                        Act.Copy, scale=DINV_s[:, t:t + 1])
                    psT = psp.tile([HID, P], bf16, tag="psT", name="psT")
                    nc.tensor.transpose(
                        psT[:], aggbf[:, t * HID:(t + 1) * HID], ident_bf[:])
                    zT = ztp.tile([HID, P], bf16, tag="zT", name="zT")
                    nc.scalar.activation(zT[:], psT[:], Act.Copy)
                    h2T = psp.tile([2 * HID, P], f32, tag="h2T", name="h2T")
                    nc.tensor.matmul(h2T[:], lhsT=W2_s[:], rhs=zT[:],
                                     start=True, stop=True)
                    h2Tbf = ztp.tile([2 * HID, P], bf16, tag="h2Tbf",
                                     name="h2Tbf")
                    nc.scalar.activation(h2Tbf[:], h2T[:], Act.Relu,
                                         bias=B2_s[:, 0:1])
                    ps3 = psp.tile([P, HID], f32, tag="ps3", name="ps3")
                    nc.tensor.matmul(ps3[:], lhsT=h2Tbf[:], rhs=W3_s[:],
                                     start=True, stop=True)
                    nc.vector.tensor_tensor(
                        out=out_strip[:, t * HID:(t + 1) * HID],
                        in0=ps3[:],
                        in1=DINV_s[:, t:t + 1].to_broadcast([P, HID]),
                        op=Alu.mult)

            def finish_group(t0, gsz):
                nc.sync.dma_start(
                    OUTS_d[:, t0 * HID:(t0 + gsz) * HID],
                    out_strip[:, t0 * HID:(t0 + gsz) * HID])

            _emit_agg(nc, meta, TBL_d, IDX_s, Bbuf, agg_strip, SELF_s,
                      msgp, ztp, finish_tile, finish_group)
    nc.compile()
    return nc


def _prog_final(meta):
    """Launch 4: layer-3 aggregation + bias, then global max pool."""
    import concourse.mybir as mybir
    import concourse.tile as tile
    from concourse.masks import make_identity

    T, Npc, NTAB, Dp = meta["T"], meta["Npc"], meta["NTAB"], meta["Dp"]
    COLS16 = meta["COLS16"]
    ncolsB = meta["ncolsB"]
    f32 = mybir.dt.float32
    i16 = mybir.dt.int16
    Alu = mybir.AluOpType
    Act = mybir.ActivationFunctionType
    Axis = mybir.AxisListType
    nc = _mk_bass()

    bf16 = mybir.dt.bfloat16
    TBL_d = nc.dram_tensor("TBL", [NTAB, HID], f32, kind="ExternalInput")
    SELF_d = nc.dram_tensor("SELF", [P, T * HID], bf16, kind="ExternalInput")
    DINV_d = nc.dram_tensor("DINV", [P, T], f32, kind="ExternalInput")
    IDX_d = nc.dram_tensor("IDX16", [P, COLS16], i16, kind="ExternalInput")
    PIDX_d = nc.dram_tensor("PIDX", [P, 8 * Dp], i16, kind="ExternalInput")
    OUT_d = nc.dram_tensor("OUT", [HID, GPC], f32, kind="ExternalOutput")

    ptbl = nc.dram_tensor("ptbl", [1 + P * T, HID], f32)

    with tile.TileContext(nc, num_cores=C) as tc:
        with (
            tc.tile_pool(name="const", bufs=1) as const,
            tc.tile_pool(name="msg", bufs=3) as msgp,
            tc.tile_pool(name="zt", bufs=4) as ztp,
            tc.tile_pool(name="psum", bufs=2, space="PSUM") as psp,
        ):
            IDX_s = const.tile([P, COLS16], i16)
            if ncolsB:
                nc.sync.dma_start(IDX_s[:, 0:8 * ncolsB],
                                  IDX_d[:, 0:8 * ncolsB])
                nc.sync.dma_start(IDX_s[:, 8 * ncolsB:],
                                  IDX_d[:, 8 * ncolsB:])
            else:
                nc.sync.dma_start(IDX_s[:], IDX_d[:])
            DINV_s = const.tile([P, T], f32)
            nc.sync.dma_start(DINV_s[:], DINV_d[:])
            SELF_s = const.tile([P, T * HID], bf16)
            nc.sync.dma_start(SELF_s[:], SELF_d[:])
            PIDX_s = const.tile([P, 8 * Dp], i16)
            nc.sync.dma_start(PIDX_s[:], PIDX_d[:])
            ident = const.tile([P, P], f32)
            make_identity(nc, ident[:])
            nirow = const.tile([1, HID], f32)
            nc.vector.memset(nirow[:], float("-inf"))
            nc.sync.dma_start(ptbl[0:1, :], nirow[:])
            agg_strip = const.tile([P, T * HID], f32)
            out_strip = const.tile([P, T * HID], f32)
            Bbuf = const.tile([P, max(ncolsB, 1) * HID], f32)

            ptbl_rows = ptbl[1:1 + P * T, :].rearrange("(p t) f -> p t f",
                                                       p=P)

            def finish_tile(t, sl):
                nc.scalar.activation(
                    out_strip[:, t * HID:(t + 1) * HID], sl,
                    Act.Copy, scale=DINV_s[:, t:t + 1])

            def finish_group(t0, gsz):
                nc.sync.dma_start(
                    ptbl_rows[:, t0:t0 + gsz, :],
                    out_strip[:, t0 * HID:(t0 + gsz) * HID]
                        .rearrange("p (t f) -> p t f", f=HID))

            _emit_agg(nc, meta, TBL_d, IDX_s, Bbuf, agg_strip, SELF_s,
                      msgp, ztp, finish_tile, finish_group)

            # two half-gathers: the second half's transfer overlaps the
            # first half's reduce
            Dp1 = (Dp + 1) // 2
            halves = []
            for (d0, dn) in ((0, Dp1), (Dp1, Dp - Dp1)):
                pmsg = msgp.tile([P, dn * HID], f32, tag="pmsg",
                                 name=f"pmsg{d0}")
                nc.gpsimd.dma_gather(
                    out_ap=pmsg[:].rearrange("p (d f) -> p d f", f=HID),
                    in_ap=ptbl[:],
                    idxs_ap=PIDX_s[:, 8 * d0:8 * (d0 + dn)],
                    num_idxs=P * dn,
                    num_idxs_reg=P * dn,
                    elem_size=HID,
                    single_packet=False,
                )
                pa = ztp.tile([P, HID], f32, tag=f"poolA{d0}",
                              name=f"poolA{d0}")
                nc.vector.tensor_reduce(
                    out=pa[:],
                    in_=pmsg[:].rearrange("p (d f) -> p f d", f=HID),
                    axis=Axis.X,
                    op=Alu.max,
                )
                halves.append(pa)
            poolA = ztp.tile([P, HID], f32, tag="poolA")
            nc.vector.tensor_tensor(out=poolA[:], in0=halves[0][:],
                                    in1=halves[1][:], op=Alu.max)
            psP = psp.tile([HID, P], f32, tag="psT")
            nc.tensor.transpose(psP[:], poolA[:], ident[:])
            poolT = ztp.tile([HID, P], f32, tag="poolT")
            nc.scalar.activation(poolT[:], psP[:], Act.Copy)
            outsb = ztp.tile([HID, GPC], f32, tag="outsb")
            pt = poolT[:].rearrange("p (g two) -> p g two", two=2)
            nc.vector.tensor_tensor(out=outsb[:], in0=pt[:, :, 0],
                                    in1=pt[:, :, 1], op=Alu.max)
            nc.sync.dma_start(OUT_d[:], outsb[:])
    nc.compile()
    return nc


# --------------------------------------------------------------------------
# Entry point
# --------------------------------------------------------------------------

_RUN_KWARGS = {}
_EXEC_NS = []    # per-launch HW exec times when tracing enabled
_PROFILE = False


def _destripe(strip, T):
    """[128, T*HID] strip -> [Npc, HID] rows."""
    return strip.reshape(P, T, HID).transpose(1, 0, 2).reshape(T * P, HID)


def _mk_tables(strips, prep, meta):
    """Per-core launch-output strips -> per-core hot/cold-ordered tables."""
    T = meta["T"]
    NTAB = meta["NTAB"]
    t_full = np.zeros((N_NODES, HID), np.float32)
    for c in range(C):
        rows = _destripe(np.asarray(strips[c]).astype(np.float32), T)
        valid = prep["rowmaps"][c] >= 0
        t_full[prep["rowmaps"][c][valid]] = rows[valid]
    tabs = []
    for c in range(C):
        tab = np.zeros((NTAB, HID), np.float32)
        tab[1:1 + N_NODES] = t_full[prep["tabperms"][c]]
        tabs.append(tab)
    return tabs


def kernel(data, edge_index, batch, W1, b1, W2, b2, W3, b3):
    import ml_dtypes
    from concourse.bass_utils import run_bass_kernel_spmd

    bf16 = ml_dtypes.bfloat16
    data = np.asarray(data, dtype=np.float32)
    edge_index = np.asarray(edge_index, dtype=np.int32)
    batch_np = np.asarray(batch, dtype=np.int32)
    W1 = np.asarray(W1, dtype=np.float32)
    b1 = np.asarray(b1, dtype=np.float32)
    W2 = np.asarray(W2, dtype=np.float32)
    b2 = np.asarray(b2, dtype=np.float32)
    W3 = np.asarray(W3, dtype=np.float32)
    b3 = np.asarray(b3, dtype=np.float32)

    prep = _host_prep(edge_index, batch_np)
    meta = prep["meta"]
    T, Npc = meta["T"], meta["Npc"]
    s = prep["s"]

    cores = list(range(C))
    del _EXEC_NS[:]

    def run(nc, in_maps):
        if _PROFILE:
            from concourse.timeline_sim import TimelineSim
            _EXEC_NS.append(TimelineSim(nc, require_finite=False).simulate())
        res = run_bass_kernel_spmd(nc, in_maps, cores, **_RUN_KWARGS)
        if res.exec_time_ns is not None:
            _EXEC_NS.append(res.exec_time_ns)
        return res.results

    # strips of per-(p,t) values b/s^power for the bias folds
    def bias_fold(bvec, power):
        out = np.zeros((C, P, T * HID), np.float32)
        for c in range(C):
            pad = np.zeros(Npc, np.float64)
            valid = prep["rowmaps"][c] >= 0
            pad[valid] = s[prep["rowmaps"][c][valid]]
            cell = pad.reshape(T, P).T                    # [P, T]
            with np.errstate(divide="ignore"):
                f = np.where(cell > 0, 1.0 / (cell ** power), 0.0)
            out[c] = (f[:, :, None] * bvec[None, None, :]).reshape(P, T * HID)
        return out

    # ---- launch 1: T1 strips ----
    nc1 = _prog_tables(meta)
    xts = []
    for c in range(C):
        xt = np.zeros((IN_DIM, Npc), np.float32)
        valid = prep["rowmaps"][c] >= 0
        nodes = prep["rowmaps"][c][valid]
        xt[:, valid] = (data[nodes] * s[nodes][:, None]).T
        xts.append(xt.astype(bf16))
    W1b = W1.astype(bf16)
    r1 = run(nc1, [{"XT": xts[c], "W1": W1b} for c in range(C)])
    s1 = [np.asarray(r1[c]["OUTS"]).astype(np.float32) for c in range(C)]
    tabs1 = _mk_tables(s1, prep, meta)

    # ---- launch 2: layer 1 -> T2 strips ----
    nc2 = _prog_layer(meta, 1)
    fold1 = bias_fold(b1, 2)
    r2 = run(nc2, [{"TBL": tabs1[c],
                    "SELF": (s1[c] + fold1[c]).astype(bf16),
                    "DINV": np.ascontiguousarray(prep["dinvT"][1, c]),
                    "IDX16": np.ascontiguousarray(prep["idx16"][c])}
                   for c in range(C)])
    s2 = [np.asarray(r2[c]["OUTS"]).astype(np.float32) for c in range(C)]
    tabs2 = _mk_tables(s2, prep, meta)

    # ---- launch 3: layer 2 -> T3 strips ----
    nc3 = _prog_layer(meta, 2)
    W2b = W2.astype(bf16)
    W3b = W3.astype(bf16)
    B2col = b2.reshape(P, 1).astype(np.float32)
    r3 = run(nc3, [{"TBL": tabs2[c],
                    "SELF": s2[c].astype(bf16),
                    "DINV": np.ascontiguousarray(prep["dinvT"][0, c]),
                    "IDX16": np.ascontiguousarray(prep["idx16"][c]),
                    "W2": W2b, "W3": W3b, "B2": B2col}
                   for c in range(C)])
    s3 = [np.asarray(r3[c]["OUTS"]).astype(np.float32) for c in range(C)]
    tabs3 = _mk_tables(s3, prep, meta)

    # ---- launch 4: layer 3 + pool ----
    nc4 = _prog_final(meta)
    fold3 = bias_fold(b3, 1)
    r4 = run(nc4, [{"TBL": tabs3[c],
                    "SELF": (s3[c] + fold3[c]).astype(bf16),
                    "DINV": np.ascontiguousarray(prep["dinvT"][0, c]),
                    "IDX16": np.ascontiguousarray(prep["idx16"][c]),
                    "PIDX": np.ascontiguousarray(prep["pool16"][c])}
                   for c in range(C)])
    out = np.concatenate(
        [np.asarray(r4[c]["OUT"]).T for c in range(C)], axis=0
    )
    return out.astype(np.float32)
